# revision 2
# baseline (speedup 1.0000x reference)
"""Trainium2 Bass kernel for nn_AttentionBlockDraft (GNN message passing).

Strategy: 1D-partition the 400k edges (idx_i sorted) across 8 cores.
Node q/k/v projections are packed into two bf16 gather tables; each core
runs the per-edge filter MLPs + attention + segment-sum scatter on device:
  - edge filter MLPs as bf16 PE matmuls (feature-major via PE transposes)
  - k/v/ev and q/ev per-edge rows via indirect-DMA gathers
  - alpha = sum(q*w*k)*cut on DVE (fused cut via scalar_tensor_tensor)
  - segment-sum scatter via selection-matrix matmuls accumulating in PSUM
    over 128-node groups (edges padded so groups align per-core)
Outputs are per-core node windows, summed on host.
"""
import numpy as np
import ml_dtypes

import concourse.bass as bass
import concourse.bacc as bacc
import concourse.tile as tile
from concourse import mybir
from concourse.bass_utils import run_bass_kernel_spmd
from concourse.masks import make_identity

N = 25000
P_EDGES = 400000
F = 224
KRBF = 32
TH = 7
FH = 32
H = 4
DV = 56
NO = 15
NCORES = 8
EPC = P_EDGES // NCORES
OFFS = [0, 3, 8, 15]
ROW_KV = 464   # [k 224 | v 224 | ev 15 | pad 1]
ROW_Q = 240    # [q 224 | ev 15 | pad 1]

bf = mybir.dt.bfloat16
f32 = mybir.dt.float32
i32 = mybir.dt.int32


def _build(E_pad, chunk_group, G, W):
    nchunks = E_pad // 128
    ntiles = E_pad // 512
    # first/last chunk of each group
    gfirst = {}
    glast = {}
    for ci, g in enumerate(chunk_group):
        if g not in gfirst:
            gfirst[g] = ci
        glast[g] = ci

    nc = bacc.Bacc("TRN2", target_bir_lowering=False)
    t_kv = nc.dram_tensor("t_kv", [N, ROW_KV], bf, kind="ExternalInput")
    t_q = nc.dram_tensor("t_q", [N, ROW_Q], bf, kind="ExternalInput")
    rbf_p = nc.dram_tensor("rbf_p", [E_pad, KRBF], bf, kind="ExternalInput")
    ylm_p = nc.dram_tensor("ylm_p", [E_pad, NO], bf, kind="ExternalInput")
    cut_p = nc.dram_tensor("cut_p", [E_pad, 1], f32, kind="ExternalInput")
    idxj_p = nc.dram_tensor("idxj_p", [E_pad, 1], i32, kind="ExternalInput")
    idxia_p = nc.dram_tensor("idxia_p", [E_pad, 1], i32, kind="ExternalInput")
    idxloc_p = nc.dram_tensor("idxloc_p", [E_pad, 1], i32, kind="ExternalInput")
    w1r_d = nc.dram_tensor("w1r_d", [KRBF, 112], bf, kind="ExternalInput")
    b1r_d = nc.dram_tensor("b1r_d", [112, 1], f32, kind="ExternalInput")
    w1s_d = nc.dram_tensor("w1s_d", [3, 112], bf, kind="ExternalInput")
    b1s_d = nc.dram_tensor("b1s_d", [112, 1], f32, kind="ExternalInput")
    w2a_d = nc.dram_tensor("w2a_d", [113, 224], bf, kind="ExternalInput")
    w2b_d = nc.dram_tensor("w2b_d", [112, 224], bf, kind="ExternalInput")
    outx = nc.dram_tensor("outx", [W, 224], f32, kind="ExternalOutput")
    oute = nc.dram_tensor("oute", [W, NO], f32, kind="ExternalOutput")

    with tile.TileContext(nc) as tc:
        with (
            tc.tile_pool(name="cst", bufs=1) as cst,
            tc.tile_pool(name="sb", bufs=2) as sb,
            tc.tile_pool(name="ps", bufs=1, space="PSUM") as ps,
            tc.tile_pool(name="psw", bufs=2, space="PSUM") as psw,
            tc.tile_pool(name="psacc", bufs=1, space="PSUM") as psacc,
        ):
            # constants
            iota_t = cst.tile([128, 128], i32)
            nc.gpsimd.iota(iota_t[:], pattern=[[1, 128]], base=0,
                           channel_multiplier=0)
            ident = cst.tile([128, 128], bf)
            make_identity(nc, ident[:])
            w1r_t = cst.tile([KRBF, 112], bf)
            nc.sync.dma_start(out=w1r_t[:], in_=w1r_d[:])
            b1r_t = cst.tile([112, 1], f32)
            nc.sync.dma_start(out=b1r_t[:], in_=b1r_d[:])
            w1s_t = cst.tile([3, 112], bf)
            nc.sync.dma_start(out=w1s_t[:], in_=w1s_d[:])
            b1s_t = cst.tile([112, 1], f32)
            nc.sync.dma_start(out=b1s_t[:], in_=b1s_d[:])
            w2a_t = cst.tile([113, 224], bf)
            nc.sync.dma_start(out=w2a_t[:], in_=w2a_d[:])
            w2b_t = cst.tile([112, 224], bf)
            nc.sync.dma_start(out=w2b_t[:], in_=w2b_d[:])

            acc_x = None
            acc_ev = None

            for t in range(ntiles):
                e0 = t * 512
                # ---- loads ----
                rbft = sb.tile([128, 4, KRBF], bf, name=f"rbft")
                nc.sync.dma_start(
                    out=rbft[:],
                    in_=rbf_p[e0:e0 + 512, :].rearrange("(c p) f -> p c f", p=128))
                ylmt = sb.tile([128, 4, NO], bf, name=f"ylmt")
                nc.sync.dma_start(
                    out=ylmt[:],
                    in_=ylm_p[e0:e0 + 512, :].rearrange("(c p) f -> p c f", p=128))
                cutt = sb.tile([128, 4], f32, name=f"cutt")
                nc.sync.dma_start(
                    out=cutt[:],
                    in_=cut_p[e0:e0 + 512, :].rearrange("(c p) o -> p (c o)", p=128))
                idxjt = sb.tile([128, 4], i32, name=f"idxjt")
                nc.sync.dma_start(
                    out=idxjt[:],
                    in_=idxj_p[e0:e0 + 512, :].rearrange("(c p) o -> p (c o)", p=128))
                idxiat = sb.tile([128, 4], i32, name=f"idxiat")
                nc.sync.dma_start(
                    out=idxiat[:],
                    in_=idxia_p[e0:e0 + 512, :].rearrange("(c p) o -> p (c o)", p=128))
                idxloct = sb.tile([128, 4], i32, name=f"idxloct")
                nc.sync.dma_start(
                    out=idxloct[:],
                    in_=idxloc_p[e0:e0 + 512, :].rearrange("(c p) o -> p (c o)", p=128))

                # ---- gathers ----
                kvt = sb.tile([128, 4, ROW_KV], bf, name=f"kvt")
                qt = sb.tile([128, 4, ROW_Q], bf, name=f"qt")
                for c in range(4):
                    nc.gpsimd.indirect_dma_start(
                        out=kvt[:, c, :], out_offset=None, in_=t_kv[:],
                        in_offset=bass.IndirectOffsetOnAxis(
                            ap=idxjt[:, c:c + 1], axis=0))
                    nc.gpsimd.indirect_dma_start(
                        out=qt[:, c, :], out_offset=None, in_=t_q[:],
                        in_offset=bass.IndirectOffsetOnAxis(
                            ap=idxiat[:, c:c + 1], axis=0))

                # ---- radial MLP ----
                rbfs = sb.tile([128, 4, KRBF], bf, name=f"rbfs")
                for c in range(4):
                    nc.vector.tensor_scalar_mul(
                        out=rbfs[:, c, :], in0=rbft[:, c, :],
                        scalar1=cutt[:, c:c + 1])
                rbfT_ps = ps.tile([KRBF, 512], bf, name="rbfT_ps")
                for c in range(4):
                    nc.tensor.transpose(
                        rbfT_ps[:, 128 * c:128 * (c + 1)], rbfs[:, c, :], ident[:])
                rbfT = sb.tile([KRBF, 512], bf, name="rbfT")
                nc.vector.tensor_copy(out=rbfT[:], in_=rbfT_ps[:])
                h1r_ps = ps.tile([112, 512], f32, name="h1r_ps")
                nc.tensor.matmul(h1r_ps[:], lhsT=w1r_t[:], rhs=rbfT[:],
                                 start=True, stop=True)
                s_a = sb.tile([113, 512], bf, name="s_a")
                nc.vector.memset(s_a[:], 1.0)
                nc.scalar.activation(out=s_a[0:112, :], in_=h1r_ps[:],
                                     func=mybir.ActivationFunctionType.Silu,
                                     bias=b1r_t[:, :1])

                # ---- spherical branch ----
                d_t = sb.tile([128, 4, NO], bf, name="d_t")
                nc.vector.tensor_tensor(
                    out=d_t[:], in0=kvt[:, :, 448:463], in1=qt[:, :, 224:239],
                    op=mybir.AluOpType.subtract)
                d2_t = sb.tile([128, 4, NO], bf, name="d2_t")
                nc.vector.tensor_tensor(out=d2_t[:], in0=d_t[:], in1=d_t[:],
                                        op=mybir.AluOpType.mult)
                l0_t = sb.tile([128, 4, 3], f32, name="l0_t")
                for dg in range(3):
                    nc.vector.tensor_reduce(
                        out=l0_t[:, :, dg:dg + 1],
                        in_=d2_t[:, :, OFFS[dg]:OFFS[dg + 1]],
                        axis=mybir.AxisListType.X, op=mybir.AluOpType.add)
                l0b = sb.tile([128, 4, 3], bf, name="l0b")
                nc.vector.tensor_copy(out=l0b[:], in_=l0_t[:])
                l0T_ps = ps.tile([3, 512], bf, name="l0T_ps")
                for c in range(4):
                    nc.tensor.transpose(
                        l0T_ps[:, 128 * c:128 * (c + 1)], l0b[:, c, :], ident[:])
                l0T = sb.tile([3, 512], bf, name="l0T")
                nc.vector.tensor_copy(out=l0T[:], in_=l0T_ps[:])
                h1s_ps = ps.tile([112, 512], f32, name="h1s_ps")
                nc.tensor.matmul(h1s_ps[:], lhsT=w1s_t[:], rhs=l0T[:],
                                 start=True, stop=True)
                s_b = sb.tile([112, 512], bf, name="s_b")
                nc.scalar.activation(out=s_b[:], in_=h1s_ps[:],
                                     func=mybir.ActivationFunctionType.Silu,
                                     bias=b1s_t[:, :1])

                # ---- second layer + attention + scatter, per chunk ----
                wsb = sb.tile([128, 4, 224], bf, name="wsb")
                for c in range(4):
                    w_ps = psw.tile([128, 224], f32, name="w_ps", tag="w_ps")
                    cs = slice(128 * c, 128 * (c + 1))
                    nc.tensor.matmul(w_ps[:], lhsT=s_a[:, cs], rhs=w2a_t[:],
                                     start=True, stop=False)
                    nc.tensor.matmul(w_ps[:], lhsT=s_b[:, cs], rhs=w2b_t[:],
                                     start=False, stop=True)
                    nc.scalar.copy(out=wsb[:, c, :], in_=w_ps[:])

                p_t = sb.tile([128, 4, 224], bf, name="p_t")
                for c in range(4):
                    nc.vector.scalar_tensor_tensor(
                        out=p_t[:, c, :], in0=qt[:, c, 0:224],
                        scalar=cutt[:, c:c + 1], in1=kvt[:, c, 0:224],
                        op0=mybir.AluOpType.mult, op1=mybir.AluOpType.mult)
                tt = sb.tile([128, 4, 224], bf, name="tt")
                nc.vector.tensor_tensor(out=tt[:], in0=p_t[:], in1=wsb[:],
                                        op=mybir.AluOpType.mult)
                alpha = sb.tile([128, 4, TH], f32, name="alpha")
                nc.vector.tensor_reduce(
                    out=alpha[:],
                    in_=tt[:].rearrange("p c (h f) -> p c h f", h=TH),
                    axis=mybir.AxisListType.X, op=mybir.AluOpType.add)
                alphab = sb.tile([128, 4, TH], bf, name="alphab")
                nc.vector.tensor_copy(out=alphab[:], in_=alpha[:])

                av = sb.tile([128, 4, 224], bf, name="av")
                for c in range(4):
                    nc.vector.tensor_tensor(
                        out=av[:, c, :].rearrange("p (h f) -> p h f", h=H),
                        in0=kvt[:, c, 224:448].rearrange("p (h f) -> p h f", h=H),
                        in1=alphab[:, c, 0:H, None].to_broadcast([128, H, DV]),
                        op=mybir.AluOpType.mult)
                a2r = sb.tile([128, 4, NO], bf, name="a2r")
                for dg in range(3):
                    rep = OFFS[dg + 1] - OFFS[dg]
                    nc.vector.tensor_copy(
                        out=a2r[:, :, OFFS[dg]:OFFS[dg + 1]],
                        in_=alphab[:, :, H + dg:H + dg + 1].to_broadcast(
                            [128, 4, rep]))
                ym = sb.tile([128, 4, NO], bf, name="ym")
                nc.vector.tensor_tensor(out=ym[:], in0=a2r[:], in1=ylmt[:],
                                        op=mybir.AluOpType.mult)

                for c in range(4):
                    ci = t * 4 + c
                    g = chunk_group[ci]
                    first = (gfirst[g] == ci)
                    last = (glast[g] == ci)
                    if first:
                        acc_x = psacc.tile([128, 224], f32, name="acc_x",
                                           tag="acc_x")
                        acc_ev = psacc.tile([128, NO], f32, name="acc_ev",
                                            tag="acc_ev")
                    smat = sb.tile([128, 128], bf, name="smat")
                    nc.vector.tensor_tensor(
                        out=smat[:], in0=idxloct[:, c:c + 1].to_broadcast([128, 128]),
                        in1=iota_t[:], op=mybir.AluOpType.is_equal)
                    nc.tensor.matmul(acc_x[:], lhsT=smat[:], rhs=av[:, c, :],
                                     start=first, stop=last)
                    nc.tensor.matmul(acc_ev[:], lhsT=smat[:], rhs=ym[:, c, :],
                                     start=first, stop=last)
                    if last:
                        fx = sb.tile([128, 224], f32, name="fx")
                        nc.scalar.copy(out=fx[:], in_=acc_x[:])
                        nc.sync.dma_start(
                            out=outx[128 * g:128 * (g + 1), :], in_=fx[:])
                        fe = sb.tile([128, NO], f32, name="fe")
                        nc.scalar.copy(out=fe[:], in_=acc_ev[:])
                        nc.sync.dma_start(
                            out=oute[128 * g:128 * (g + 1), :], in_=fe[:])
    nc.compile()
    return nc


def _host_prep(x, ev, rbf_ij, ylm_ij, cut, idx_i, idx_j,
               W1r, b1r, W2r, b2r, W1s, b1s, W2s, b2s, Wq, Wk, Wv):
    def silu(z):
        return z / (1.0 + np.exp(-z))

    xH = x.reshape(N, TH, FH)
    q = silu(np.einsum('Hij,NHj->NHi', Wq, xH)).reshape(N, F)
    k = silu(np.einsum('Hij,NHj->NHi', Wk, xH)).reshape(N, F)
    xh = x.reshape(N, H, DV)
    v = np.einsum('hij,Nhj->Nhi', Wv, xh).reshape(N, F)

    t_kv = np.zeros((N, ROW_KV), dtype=ml_dtypes.bfloat16)
    t_kv[:, 0:224] = k
    t_kv[:, 224:448] = v
    t_kv[:, 448:463] = ev
    t_q = np.zeros((N, ROW_Q), dtype=ml_dtypes.bfloat16)
    t_q[:, 0:224] = q
    t_q[:, 224:239] = ev

    # per-core edge shards
    ebnd = [c * EPC for c in range(NCORES + 1)]
    lo = [int(idx_i[ebnd[c]]) for c in range(NCORES)]
    hi = [int(idx_i[ebnd[c + 1] - 1]) for c in range(NCORES)]
    Wn = max(hi[c] - lo[c] + 1 for c in range(NCORES))
    G = (Wn + 127) // 128
    Wwin = G * 128

    # group counts per core
    counts = np.zeros((NCORES, G), dtype=np.int64)
    rel = [None] * NCORES
    for c in range(NCORES):
        r = idx_i[ebnd[c]:ebnd[c + 1]] - lo[c]
        rel[c] = r
        counts[c] = np.bincount(r // 128, minlength=G)
    gsize = ((counts.max(axis=0) + 127) // 128 * 128).astype(np.int64)
    E_pad = int(gsize.sum())
    if E_pad % 512 != 0:
        extra = 512 - E_pad % 512
        gsize[-1] += extra
        E_pad += extra

    chunk_group = []
    for g in range(G):
        chunk_group += [g] * (int(gsize[g]) // 128)

    in_maps = []
    for c in range(NCORES):
        rbf_c = np.zeros((E_pad, KRBF), dtype=ml_dtypes.bfloat16)
        ylm_c = np.zeros((E_pad, NO), dtype=ml_dtypes.bfloat16)
        cut_c = np.zeros((E_pad, 1), dtype=np.float32)
        idxj_c = np.zeros((E_pad, 1), dtype=np.int32)
        idxia_c = np.zeros((E_pad, 1), dtype=np.int32)
        idxloc_c = np.full((E_pad, 1), 127, dtype=np.int32)
        e0, e1 = ebnd[c], ebnd[c + 1]
        r = rel[c]
        gidx = r // 128
        order = np.argsort(gidx, kind="stable")
        src = np.arange(e0, e1)[order]
        gsorted = gidx[order]
        # destination offsets: groups packed at cumulative gsize offsets
        goff = np.concatenate([[0], np.cumsum(gsize)]).astype(np.int64)
        dst = np.empty(e1 - e0, dtype=np.int64)
        pos = 0
        for g in range(G):
            ng = int(counts[c, g])
            dst[pos:pos + ng] = goff[g] + np.arange(ng)
            pos += ng
        rbf_c[dst] = rbf_ij[src].astype(ml_dtypes.bfloat16)
        ylm_c[dst] = ylm_ij[src].astype(ml_dtypes.bfloat16)
        cut_c[dst, 0] = cut[src]
        idxj_c[dst, 0] = idx_j[src]
        idxia_c[dst, 0] = idx_i[src]
        idxloc_c[dst, 0] = (idx_i[src] - lo[c]) % 128
        in_maps.append({
            "t_kv": t_kv, "t_q": t_q, "rbf_p": rbf_c, "ylm_p": ylm_c,
            "cut_p": cut_c, "idxj_p": idxj_c, "idxia_p": idxia_c,
            "idxloc_p": idxloc_c,
            "w1r_d": W1r.astype(ml_dtypes.bfloat16),
            "b1r_d": b1r.reshape(112, 1).astype(np.float32),
            "w1s_d": W1s.astype(ml_dtypes.bfloat16),
            "b1s_d": b1s.reshape(112, 1).astype(np.float32),
            "w2a_d": np.concatenate(
                [W2r, (b2r + b2s).reshape(1, 224)], axis=0).astype(
                    ml_dtypes.bfloat16),
            "w2b_d": W2s.astype(ml_dtypes.bfloat16),
        })
    return in_maps, chunk_group, G, Wwin, E_pad, lo


_CACHE = {}


def kernel(x, ev, rbf_ij, ylm_ij, cut, idx_i, idx_j,
           W1r, b1r, W2r, b2r, W1s, b1s, W2s, b2s, Wq, Wk, Wv):
    args = [np.asarray(a) for a in (x, ev, rbf_ij, ylm_ij, cut, idx_i, idx_j,
                                    W1r, b1r, W2r, b2r, W1s, b1s, W2s, b2s,
                                    Wq, Wk, Wv)]
    in_maps, chunk_group, G, Wwin, E_pad, lo = _host_prep(*args)
    key = (E_pad, tuple(chunk_group), G, Wwin)
    if key not in _CACHE:
        _CACHE[key] = _build(E_pad, chunk_group, G, Wwin)
    nc = _CACHE[key]
    res = run_bass_kernel_spmd(nc, in_maps, core_ids=list(range(NCORES)))
    x_att = np.zeros((N, F), dtype=np.float32)
    ev_att = np.zeros((N, NO), dtype=np.float32)
    for c in range(NCORES):
        n0 = lo[c]
        n1 = min(n0 + Wwin, N)
        x_att[n0:n1] += res.results[c]["outx"][:n1 - n0]
        ev_att[n0:n1] += res.results[c]["oute"][:n1 - n0]
    return (x_att, ev_att)
